# revision 36
# baseline (speedup 1.0000x reference)
"""Trainium2 Bass kernel for GNN message passing (nn_Kernel_17772574670927).

Reference computes, per node b with N=8 neighbors:
    out[b] = sum_n concat(node_v[b], node_h[b], nbr_v[b,n], nbr_h[b,n]) @ W + bias
Since the linear layer distributes over the neighbor sum:
    out[b] = (N*node_v[b])*W[0] + (N*node_h[b]) @ W[1:257]
           + (sum_n nbr_v[b,n])*W[257] + (sum_n nbr_h[b,n]) @ W[258:514] + N*bias
an 8x FLOP reduction vs the naive form; the kernel is then HBM-bound on
streaming nbr_h.

v11: the host reshapes the two big streams into the bf16 feature-major
layout the tensor engine wants (a pure cast+layout change -- the kernel
already consumed both tensors in bf16, the rounding just moves before
the first add):
    nbrT   [2, 128, B, 8]  nbrT[k, f, b, n]  = bf16(nbr_h[b, n, 128k+f])
    nodehT [2, 128, B]     nodehT[k, f, b]   = bf16(node_h[b, 128k+f])
On device, per 128-node tile, the whole pipeline collapses to:
    - one contiguous DVE tensor_reduce over n -> feature-major nbr sum
      (which IS the matmul lhsT -- no PE transposes, no PSUM staging)
    - five PSUM-accumulating matmuls (2 node_h blocks, 2 nbr-sum blocks,
      K=3 v/bias block)
    - one ACT psum->bf16 copy + write
Streams: sync HWDGE ring carries only nbrT (1 MiB granules, 10
buffered); scalar HWDGE ring carries nodehT chunks + setup + output
writes; gpsimd and PE transposes are gone entirely.

Small host-prepped constants (as before):
    wb  [4, 128, H]: N*W[1:129], N*W[129:257], W[258:386], W[386:514]
    vw  [3, H]:      N*W[0], N*b, W[257]
    nv3 [3, BP]:     node_v, ones, sum_n nbr_v   (K=3 lhsT columns)

Sharding: data-parallel over the node dim B=65536 across 8 cores (8192 each).
Weights replicated. No collectives.
"""
import sys

for _p in ("/root/.axon_site", "/root/.axon_site/_ro/trn_rl_repo", "/opt/trn_rl_repo"):
    if _p not in sys.path:
        sys.path.append(_p)

import ml_dtypes
import numpy as np

import concourse.bacc as bacc
import concourse.bass as bass
import concourse.mybir as mybir
from concourse.tile import TileContext

B, N, H = 65536, 8, 256
NCORES = 8
BP = B // NCORES          # 8192 nodes per core
P = 128                   # SBUF partitions
NTILES = BP // P          # 64 node-tiles per core
RG = 4                    # tiles per nbrT read granule (two 1 MiB DMAs)
NGRAN = NTILES // RG
CHUNK = 8                 # tiles per nodehT chunk (512 KiB)
NCHUNKS = NTILES // CHUNK
F32 = mybir.dt.float32
BF16 = mybir.dt.bfloat16
BF16NP = ml_dtypes.bfloat16


def build_bass() -> bass.Bass:
    nc = bacc.Bacc("TRN2", target_bir_lowering=False, debug=False,
                   num_devices=NCORES)
    nbrT = nc.dram_tensor("nbrT", [2, P, BP // 512, N, 512], BF16, kind="ExternalInput")
    nodehT = nc.dram_tensor("nodehT", [2, P, BP], BF16, kind="ExternalInput")
    wb = nc.dram_tensor("wb", [4, P, H], BF16, kind="ExternalInput")
    vw = nc.dram_tensor("vw", [3, H], BF16, kind="ExternalInput")
    nv3 = nc.dram_tensor("nv3", [3, BP], BF16, kind="ExternalInput")
    # bf16 output halves the write traffic; host upcasts to fp32 after
    # gather.  4-tile-packed blocked layout [g4, p, u, h] makes each
    # output write a 256 KB DMA with 2 KiB descriptors (vs 512 B rows).
    out = nc.dram_tensor("out", [NTILES // 4, P, 4, H], BF16,
                         kind="ExternalOutput")

    with TileContext(nc) as tc, nc.allow_low_precision(
        reason="bf16 GEMM inputs; harness tolerance is 2e-2"
    ):
        with (
            tc.tile_pool(name="singles", bufs=1) as singles,
            tc.tile_pool(name="nbr", bufs=10) as nbr_pool,
            tc.tile_pool(name="a1", bufs=3) as a1_pool,
            tc.tile_pool(name="a2", bufs=3) as a2_pool,
            tc.tile_pool(name="a3", bufs=4) as a3_pool,
            tc.tile_pool(name="nodeh", bufs=3) as nodeh_pool,
            tc.tile_pool(name="outp", bufs=3) as out_pool,
            tc.tile_pool(name="pout", bufs=6, space="PSUM") as psum_out_pool,
        ):
            # ---- nbrT stream: all on the sync HWDGE ring, nothing else
            # there.  [P, RG, 2, P, N]: partition = feature-within-block,
            # 2 KiB descriptors, 1 MiB per DMA.
            # last granule tapers into 2-tile halves so the final tiles'
            # compute starts ~4us sooner instead of landing all at once
            nbr_tiles = []
            for g in range(NGRAN):
                nb = nbr_pool.tile([P, 2, N, RG * P], BF16, tag="nbr")
                if g < NGRAN - 1:
                    for k in range(2):
                        nc.sync.dma_start(
                            out=nb[:, k], in_=nbrT[k, :, g, :, :]
                        )
                else:
                    for half in range(2):
                        qs = slice(half * 2 * P, (half + 1) * 2 * P)
                        for k in range(2):
                            nc.sync.dma_start(
                                out=nb[:, k, :, qs],
                                in_=nbrT[k, :, g, :, qs],
                            )
                nbr_tiles.append(nb)

            # ---- nodehT chunks on the scalar ring: [P, 2, CHUNK*P] bf16,
            # 2 KiB descriptors, always-ready
            nodeh_chunks = []
            for c in range(NCHUNKS):
                ph = nodeh_pool.tile([P, 2, CHUNK * P], BF16, tag="nodeh")
                nc.scalar.dma_start(
                    out=ph[:],
                    in_=nodehT[:, :, c * CHUNK * P:(c + 1) * CHUNK * P].rearrange(
                        "k f q -> f k q"
                    ),
                )
                nodeh_chunks.append(ph)

            # ---- one-time setup: three small bf16 loads on the scalar ring
            w_sb = singles.tile([P, 4, H], BF16)
            nc.scalar.dma_start(
                out=w_sb[:], in_=wb.rearrange("c p h -> p c h")
            )
            v_w3 = singles.tile([3, H], BF16)
            nc.scalar.dma_start(out=v_w3[:], in_=vw[:, :])
            vcolsT = singles.tile([3, NTILES, P], BF16)
            nc.scalar.dma_start(
                out=vcolsT[:], in_=nv3.rearrange("c (t p) -> c t p", p=P)
            )

            # ---- main loop ----
            for t in range(NTILES):
                g, u = divmod(t, RG)
                c, j = divmod(t, CHUNK)
                nbr_tile = nbr_tiles[g]
                nodeh_chunk = nodeh_chunks[c]

                # neighbor sum: 3-level contiguous bf16 add tree, split
                # by k-block across DVE (k=0) and GpSimd (k=1) -> [P, 2, P]
                # feature-major, directly usable as lhsT
                qs = slice(u * P, (u + 1) * P)
                a1 = a1_pool.tile([P, 2, 4, P], BF16)
                a2 = a2_pool.tile([P, 2, 2, P], BF16)
                a3 = a3_pool.tile([P, 2, P], BF16)
                for k, eng in ((0, nc.vector), (1, nc.gpsimd)):
                    eng.tensor_add(
                        out=a1[:, k], in0=nbr_tile[:, k, 0:4, qs],
                        in1=nbr_tile[:, k, 4:8, qs],
                    )
                    eng.tensor_add(
                        out=a2[:, k], in0=a1[:, k, 0:2], in1=a1[:, k, 2:4]
                    )
                    eng.tensor_add(
                        out=a3[:, k], in0=a2[:, k, 0], in1=a2[:, k, 1]
                    )

                # accumulate all five K-blocks into PSUM (bias included)
                psum_out = psum_out_pool.tile([P, H], F32)
                for i in range(2):
                    nc.tensor.matmul(
                        psum_out[:],
                        nodeh_chunk[:, i, j * P:(j + 1) * P],
                        w_sb[:, i, :],
                        start=(i == 0), stop=False,
                    )
                for i in range(2):
                    nc.tensor.matmul(
                        psum_out[:], a3[:, i, :], w_sb[:, 2 + i, :],
                        start=False, stop=False,
                    )
                nc.tensor.matmul(
                    psum_out[:], vcolsT[:, t, :], v_w3[:],
                    start=False, stop=True,
                )

                # bf16 writes on the scalar ring: packed 4 tiles per DMA
                # (2 KiB descriptors); the last granule writes per-tile so
                # the tail doesn't wait on a full pack
                g4, u4 = divmod(t, 4)
                if u4 == 0:
                    out_tile = out_pool.tile([P, 4, H], BF16, tag="out4")
                nc.scalar.copy(out=out_tile[:, u4, :], in_=psum_out[:])
                if t >= NTILES - 4:
                    nc.scalar.dma_start(
                        out=out[g4, :, u4, :], in_=out_tile[:, u4, :]
                    )
                elif u4 == 3:
                    nc.scalar.dma_start(out=out[g4], in_=out_tile[:])
    nc.compile()
    return nc


_BASS_CACHE = None


def _get_bass():
    global _BASS_CACHE
    if _BASS_CACHE is None:
        _BASS_CACHE = build_bass()
    return _BASS_CACHE


def _prep_small(inputs: dict):
    """Host-side prep of the tiny pre-arranged bf16 weight tensors."""
    Wf = np.asarray(inputs["W"], dtype=np.float32)
    bf = np.asarray(inputs["b"], dtype=np.float32)
    node_v = np.asarray(inputs["node_v"], dtype=np.float32).reshape(B)
    nbr_v = np.asarray(inputs["nbr_v"], dtype=np.float32).reshape(B, N)

    wb = np.empty((4, P, H), dtype=np.float32)
    wb[0] = N * Wf[1:129]
    wb[1] = N * Wf[129:257]
    wb[2] = Wf[258:386]
    wb[3] = Wf[386:514]
    vw = np.stack([N * Wf[0], N * bf, Wf[257]], axis=0)
    nv3 = np.stack(
        [node_v, np.ones(B, dtype=np.float32), nbr_v.sum(axis=1)], axis=0
    )
    return (
        np.ascontiguousarray(wb.astype(BF16NP)),
        np.ascontiguousarray(vw.astype(BF16NP)),
        np.ascontiguousarray(nv3.astype(BF16NP)),
    )


def run_sharded(inputs: dict, trace: bool = False, trace_cores=None):
    """Shard full inputs over 8 cores, run, gather. Returns (out, results)."""
    from concourse.bass_utils import run_bass_kernel_spmd

    nc = _get_bass()
    # bf16 feature-major repack of the two big streams (pure cast+layout;
    # the kernel consumed both in bf16 on-chip anyway)
    nbr16 = np.asarray(inputs["nbr_h"], dtype=np.float32).astype(BF16NP)
    nodeh16 = np.asarray(inputs["node_h"], dtype=np.float32).astype(BF16NP)
    wb, vw, nv3 = _prep_small(inputs)

    in_maps = []
    for core in range(NCORES):
        s = slice(core * BP, (core + 1) * BP)
        nbrT = np.ascontiguousarray(
            nbr16[s].reshape(BP // 512, 512, N, 2, P).transpose(3, 4, 0, 2, 1)
        )
        nodehT = np.ascontiguousarray(
            nodeh16[s].reshape(BP, 2, P).transpose(1, 2, 0)
        )
        in_maps.append({
            "nbrT": nbrT, "nodehT": nodehT,
            "wb": wb, "vw": vw,
            "nv3": np.ascontiguousarray(nv3[:, s]),
        })
    kwargs = {}
    if trace:
        kwargs.update(trace=True, trace_cores=trace_cores or [0])
    res = run_bass_kernel_spmd(nc, in_maps, core_ids=list(range(NCORES)), **kwargs)
    full = np.concatenate(
        [
            np.asarray(res.results[i]["out"])
            .astype(np.float32)
            .transpose(0, 2, 1, 3)
            .reshape(BP, H)
            for i in range(NCORES)
        ],
        axis=0,
    )
    return full, res


def kernel(**inputs) -> np.ndarray:
    # Retry guards against the rare transient device error
    # (NRT_EXEC_UNIT_UNRECOVERABLE) seen on back-to-back runs; the compiled
    # NEFF is cached so a retry only re-executes.
    import time as _time

    last_err = None
    for attempt in range(3):
        try:
            out, _ = run_sharded(inputs, trace=False)
            return out
        except Exception as e:  # noqa: BLE001 - re-raised after retries
            last_err = e
            _time.sleep(2.0)
    raise last_err


if __name__ == "__main__":
    rng = np.random.default_rng(0)
    fake = {
        "node_v": rng.standard_normal((B, 1), dtype=np.float32),
        "node_h": rng.standard_normal((B, H), dtype=np.float32),
        "nbr_v": rng.standard_normal((B, N, 1), dtype=np.float32),
        "nbr_h": rng.standard_normal((B, N, H), dtype=np.float32),
        "W": rng.standard_normal((514, H), dtype=np.float32) / np.sqrt(514),
        "b": np.zeros((H,), dtype=np.float32),
        "iteration": 0,
    }
    got = kernel(**fake)
    sf = np.concatenate([fake["node_v"], fake["node_h"]], axis=-1)
    nf = np.concatenate([fake["nbr_v"], fake["nbr_h"]], axis=-1)
    exp = (
        N * sf @ fake["W"][:257] + nf.sum(axis=1) @ fake["W"][257:] + N * fake["b"]
    )
    err = np.abs(got - exp).max() / np.abs(exp).max()
    print("rel err vs numpy:", err)


# revision 37
# speedup vs baseline: 1.0912x; 1.0912x over previous
"""Trainium2 Bass kernel for GNN message passing (nn_Kernel_17772574670927).

Reference computes, per node b with N=8 neighbors:
    out[b] = sum_n concat(node_v[b], node_h[b], nbr_v[b,n], nbr_h[b,n]) @ W + bias
Since the linear layer distributes over the neighbor sum:
    out[b] = (N*node_v[b])*W[0] + (N*node_h[b]) @ W[1:257]
           + (sum_n nbr_v[b,n])*W[257] + (sum_n nbr_h[b,n]) @ W[258:514] + N*bias
an 8x FLOP reduction vs the naive form; the kernel is then HBM-bound on
streaming nbr_h.

v11: the host reshapes the two big streams into the bf16 feature-major
layout the tensor engine wants (a pure cast+layout change -- the kernel
already consumed both tensors in bf16, the rounding just moves before
the first add):
    nbrT   [2, 128, B, 8]  nbrT[k, f, b, n]  = bf16(nbr_h[b, n, 128k+f])
    nodehT [2, 128, B]     nodehT[k, f, b]   = bf16(node_h[b, 128k+f])
On device, per 128-node tile, the whole pipeline collapses to:
    - one contiguous DVE tensor_reduce over n -> feature-major nbr sum
      (which IS the matmul lhsT -- no PE transposes, no PSUM staging)
    - five PSUM-accumulating matmuls (2 node_h blocks, 2 nbr-sum blocks,
      K=3 v/bias block)
    - one ACT psum->bf16 copy + write
Streams: sync HWDGE ring carries only nbrT (1 MiB granules, 10
buffered); scalar HWDGE ring carries nodehT chunks + setup + output
writes; gpsimd and PE transposes are gone entirely.

Small host-prepped constants (as before):
    wb  [4, 128, H]: N*W[1:129], N*W[129:257], W[258:386], W[386:514]
    vw  [3, H]:      N*W[0], N*b, W[257]
    nv3 [3, BP]:     node_v, ones, sum_n nbr_v   (K=3 lhsT columns)

Sharding: data-parallel over the node dim B=65536 across 8 cores (8192 each).
Weights replicated. No collectives.
"""
import sys

for _p in ("/root/.axon_site", "/root/.axon_site/_ro/trn_rl_repo", "/opt/trn_rl_repo"):
    if _p not in sys.path:
        sys.path.append(_p)

import ml_dtypes
import numpy as np

import concourse.bacc as bacc
import concourse.bass as bass
import concourse.mybir as mybir
from concourse.tile import TileContext

B, N, H = 65536, 8, 256
NCORES = 8
BP = B // NCORES          # 8192 nodes per core
P = 128                   # SBUF partitions
NTILES = BP // P          # 64 node-tiles per core
RG = 4                    # tiles per nbrT read granule (two 1 MiB DMAs)
NGRAN = NTILES // RG
CHUNK = 8                 # tiles per nodehT chunk (512 KiB)
NCHUNKS = NTILES // CHUNK
F32 = mybir.dt.float32
BF16 = mybir.dt.bfloat16
BF16NP = ml_dtypes.bfloat16


def build_bass() -> bass.Bass:
    nc = bacc.Bacc("TRN2", target_bir_lowering=False, debug=False,
                   num_devices=NCORES)
    nbrT = nc.dram_tensor("nbrT", [2, P, BP // 512, N, 512], BF16, kind="ExternalInput")
    nodehT = nc.dram_tensor("nodehT", [2, P, BP], BF16, kind="ExternalInput")
    wb = nc.dram_tensor("wb", [4, P, H], BF16, kind="ExternalInput")
    vw = nc.dram_tensor("vw", [3, H], BF16, kind="ExternalInput")
    nv3 = nc.dram_tensor("nv3", [3, BP], BF16, kind="ExternalInput")
    # bf16 output halves the write traffic; host upcasts to fp32 after
    # gather.  4-tile-packed blocked layout [g4, p, u, h] makes each
    # output write a 256 KB DMA with 2 KiB descriptors (vs 512 B rows).
    out = nc.dram_tensor("out", [NTILES // 4, P, 4, H], BF16,
                         kind="ExternalOutput")

    with TileContext(nc) as tc, nc.allow_low_precision(
        reason="bf16 GEMM inputs; harness tolerance is 2e-2"
    ):
        with (
            tc.tile_pool(name="singles", bufs=1) as singles,
            tc.tile_pool(name="nbr", bufs=10) as nbr_pool,
            tc.tile_pool(name="a1", bufs=3) as a1_pool,
            tc.tile_pool(name="a2", bufs=3) as a2_pool,
            tc.tile_pool(name="a3", bufs=4) as a3_pool,
            tc.tile_pool(name="nodeh", bufs=3) as nodeh_pool,
            tc.tile_pool(name="outp", bufs=3) as out_pool,
            tc.tile_pool(name="pout", bufs=6, space="PSUM") as psum_out_pool,
        ):
            # ---- nbrT stream: all on the sync HWDGE ring, nothing else
            # there.  [P, RG, 2, P, N]: partition = feature-within-block,
            # 2 KiB descriptors, 1 MiB per DMA.
            # last granule tapers into 2-tile halves so the final tiles'
            # compute starts ~4us sooner instead of landing all at once
            nbr_tiles = []
            for g in range(NGRAN):
                nb = nbr_pool.tile([P, 2, N, RG * P], BF16, tag="nbr")
                engs = (nc.sync, nc.scalar)
                if g < NGRAN - 1:
                    for k in range(2):
                        engs[k].dma_start(
                            out=nb[:, k], in_=nbrT[k, :, g, :, :]
                        )
                else:
                    for half in range(2):
                        qs = slice(half * 2 * P, (half + 1) * 2 * P)
                        for k in range(2):
                            engs[k].dma_start(
                                out=nb[:, k, :, qs],
                                in_=nbrT[k, :, g, :, qs],
                            )
                nbr_tiles.append(nb)

            # ---- nodehT chunks on the scalar ring: [P, 2, CHUNK*P] bf16,
            # 2 KiB descriptors, always-ready
            nodeh_chunks = []
            for c in range(NCHUNKS):
                ph = nodeh_pool.tile([P, 2, CHUNK * P], BF16, tag="nodeh")
                nc.scalar.dma_start(
                    out=ph[:],
                    in_=nodehT[:, :, c * CHUNK * P:(c + 1) * CHUNK * P].rearrange(
                        "k f q -> f k q"
                    ),
                )
                nodeh_chunks.append(ph)

            # ---- one-time setup: three small bf16 loads on the scalar ring
            w_sb = singles.tile([P, 4, H], BF16)
            nc.scalar.dma_start(
                out=w_sb[:], in_=wb.rearrange("c p h -> p c h")
            )
            v_w3 = singles.tile([3, H], BF16)
            nc.scalar.dma_start(out=v_w3[:], in_=vw[:, :])
            vcolsT = singles.tile([3, NTILES, P], BF16)
            nc.scalar.dma_start(
                out=vcolsT[:], in_=nv3.rearrange("c (t p) -> c t p", p=P)
            )

            # ---- main loop ----
            for t in range(NTILES):
                g, u = divmod(t, RG)
                c, j = divmod(t, CHUNK)
                nbr_tile = nbr_tiles[g]
                nodeh_chunk = nodeh_chunks[c]

                # neighbor sum: 3-level contiguous bf16 add tree on DVE
                # (tensor_tensor uses both DVE read ports; reduce does not)
                # -> [P, 2, P] feature-major, directly usable as lhsT
                qs = slice(u * P, (u + 1) * P)
                a1 = a1_pool.tile([P, 2, 4, P], BF16)
                nc.vector.tensor_add(
                    out=a1[:], in0=nbr_tile[:, :, 0:4, qs],
                    in1=nbr_tile[:, :, 4:8, qs],
                )
                a2 = a2_pool.tile([P, 2, 2, P], BF16)
                nc.vector.tensor_add(
                    out=a2[:], in0=a1[:, :, 0:2], in1=a1[:, :, 2:4]
                )
                a3 = a3_pool.tile([P, 2, P], BF16)
                nc.vector.tensor_add(
                    out=a3[:], in0=a2[:, :, 0], in1=a2[:, :, 1]
                )

                # accumulate all five K-blocks into PSUM (bias included)
                psum_out = psum_out_pool.tile([P, H], F32)
                for i in range(2):
                    nc.tensor.matmul(
                        psum_out[:],
                        nodeh_chunk[:, i, j * P:(j + 1) * P],
                        w_sb[:, i, :],
                        start=(i == 0), stop=False,
                    )
                for i in range(2):
                    nc.tensor.matmul(
                        psum_out[:], a3[:, i, :], w_sb[:, 2 + i, :],
                        start=False, stop=False,
                    )
                nc.tensor.matmul(
                    psum_out[:], vcolsT[:, t, :], v_w3[:],
                    start=False, stop=True,
                )

                # bf16 writes on the scalar ring: packed 4 tiles per DMA
                # (2 KiB descriptors); the last granule writes per-tile so
                # the tail doesn't wait on a full pack
                g4, u4 = divmod(t, 4)
                if u4 == 0:
                    out_tile = out_pool.tile([P, 4, H], BF16, tag="out4")
                nc.scalar.copy(out=out_tile[:, u4, :], in_=psum_out[:])
                if t >= NTILES - 4:
                    nc.scalar.dma_start(
                        out=out[g4, :, u4, :], in_=out_tile[:, u4, :]
                    )
                elif u4 == 3:
                    nc.gpsimd.dma_start(out=out[g4], in_=out_tile[:])
    nc.compile()
    return nc


_BASS_CACHE = None


def _get_bass():
    global _BASS_CACHE
    if _BASS_CACHE is None:
        _BASS_CACHE = build_bass()
    return _BASS_CACHE


def _prep_small(inputs: dict):
    """Host-side prep of the tiny pre-arranged bf16 weight tensors."""
    Wf = np.asarray(inputs["W"], dtype=np.float32)
    bf = np.asarray(inputs["b"], dtype=np.float32)
    node_v = np.asarray(inputs["node_v"], dtype=np.float32).reshape(B)
    nbr_v = np.asarray(inputs["nbr_v"], dtype=np.float32).reshape(B, N)

    wb = np.empty((4, P, H), dtype=np.float32)
    wb[0] = N * Wf[1:129]
    wb[1] = N * Wf[129:257]
    wb[2] = Wf[258:386]
    wb[3] = Wf[386:514]
    vw = np.stack([N * Wf[0], N * bf, Wf[257]], axis=0)
    nv3 = np.stack(
        [node_v, np.ones(B, dtype=np.float32), nbr_v.sum(axis=1)], axis=0
    )
    return (
        np.ascontiguousarray(wb.astype(BF16NP)),
        np.ascontiguousarray(vw.astype(BF16NP)),
        np.ascontiguousarray(nv3.astype(BF16NP)),
    )


def run_sharded(inputs: dict, trace: bool = False, trace_cores=None):
    """Shard full inputs over 8 cores, run, gather. Returns (out, results)."""
    from concourse.bass_utils import run_bass_kernel_spmd

    nc = _get_bass()
    # bf16 feature-major repack of the two big streams (pure cast+layout;
    # the kernel consumed both in bf16 on-chip anyway)
    nbr16 = np.asarray(inputs["nbr_h"], dtype=np.float32).astype(BF16NP)
    nodeh16 = np.asarray(inputs["node_h"], dtype=np.float32).astype(BF16NP)
    wb, vw, nv3 = _prep_small(inputs)

    in_maps = []
    for core in range(NCORES):
        s = slice(core * BP, (core + 1) * BP)
        nbrT = np.ascontiguousarray(
            nbr16[s].reshape(BP // 512, 512, N, 2, P).transpose(3, 4, 0, 2, 1)
        )
        nodehT = np.ascontiguousarray(
            nodeh16[s].reshape(BP, 2, P).transpose(1, 2, 0)
        )
        in_maps.append({
            "nbrT": nbrT, "nodehT": nodehT,
            "wb": wb, "vw": vw,
            "nv3": np.ascontiguousarray(nv3[:, s]),
        })
    kwargs = {}
    if trace:
        kwargs.update(trace=True, trace_cores=trace_cores or [0])
    res = run_bass_kernel_spmd(nc, in_maps, core_ids=list(range(NCORES)), **kwargs)
    full = np.concatenate(
        [
            np.asarray(res.results[i]["out"])
            .astype(np.float32)
            .transpose(0, 2, 1, 3)
            .reshape(BP, H)
            for i in range(NCORES)
        ],
        axis=0,
    )
    return full, res


def kernel(**inputs) -> np.ndarray:
    # Retry guards against the rare transient device error
    # (NRT_EXEC_UNIT_UNRECOVERABLE) seen on back-to-back runs; the compiled
    # NEFF is cached so a retry only re-executes.
    import time as _time

    last_err = None
    for attempt in range(3):
        try:
            out, _ = run_sharded(inputs, trace=False)
            return out
        except Exception as e:  # noqa: BLE001 - re-raised after retries
            last_err = e
            _time.sleep(2.0)
    raise last_err


if __name__ == "__main__":
    rng = np.random.default_rng(0)
    fake = {
        "node_v": rng.standard_normal((B, 1), dtype=np.float32),
        "node_h": rng.standard_normal((B, H), dtype=np.float32),
        "nbr_v": rng.standard_normal((B, N, 1), dtype=np.float32),
        "nbr_h": rng.standard_normal((B, N, H), dtype=np.float32),
        "W": rng.standard_normal((514, H), dtype=np.float32) / np.sqrt(514),
        "b": np.zeros((H,), dtype=np.float32),
        "iteration": 0,
    }
    got = kernel(**fake)
    sf = np.concatenate([fake["node_v"], fake["node_h"]], axis=-1)
    nf = np.concatenate([fake["nbr_v"], fake["nbr_h"]], axis=-1)
    exp = (
        N * sf @ fake["W"][:257] + nf.sum(axis=1) @ fake["W"][257:] + N * fake["b"]
    )
    err = np.abs(got - exp).max() / np.abs(exp).max()
    print("rel err vs numpy:", err)
